# revision 32
# baseline (speedup 1.0000x reference)
"""AdaptiveLowPassFilter Trainium2 kernel — 8-core batch-parallel.

Per core (one image, x [96,128,128] f32):
  phase A  (PE):  fused depthwise3x3+pw1 as 9 shifted matmuls accumulating in
                  PSUM (channels on partitions, padded flat pixels free; all
                  9 taps are pure free-dim offsets).
  leaky    (ACT): bias + LeakyReLU(0.2) from PSUM into h2 (bf16).
  phase B  (PE):  pw2 computed TRANSPOSED per image row: lhsT=h2[:,row],
                  rhs=pw2^T (aug with bias row) -> logits land pixel-major
                  [w, 9] in PSUM; ACT exp -> unnormalized e_t.
  softmax  (DVE): den = reduce over 9 taps, reciprocal; kw2 = e*recip written
                  as duplicated PAIRS (so phase C multiplies hit the DVE
                  2x_1P mode: innermost AP dim is a step-1 bf16 pair).
  x_t      (PE):  per-row transposes of x into pixel-major [w, (h, c)];
                  w+-1 taps via partition-shifted SBUF->SBUF band DMAs.
  phase C  (DVE): per 16-row band: 9 big tensor_tensor pair-multiplies +
                  8 big adds: num[w,(h,c)] = sum_k kw[w,h,k]*x_t{shift}[...].
  out      (ACT): bf16->f32, DMA out as [w, (h, c)]; host transposes.

All phases are banded (16 rows) so Tile can pipeline engines end-to-end.
"""
import sys
sys.path.insert(0, "/opt/trn_rl_repo")

import numpy as np
import ml_dtypes
from contextlib import ExitStack

C, CO, H, W, K = 96, 48, 128, 128, 9
RS = 130            # padded row stride in flat pixel space
PIX0 = 131          # flat offset of pixel (0, 0)
XBF = 17160         # x_bf free size
NQ = 16896          # 33 chunks x 512 of h2 pixel space
NCHUNK = 33
CHUNK = 512
HB = 16             # band height (rows)
NB = H // HB        # 8 bands

_CACHE = {}


def _build():
    import concourse.bass as bass
    import concourse.bacc as bacc
    import concourse.tile as tile
    import concourse.mybir as mybir

    dt = mybir.dt
    f32, bf16 = dt.float32, dt.bfloat16
    AF = mybir.ActivationFunctionType
    OP = mybir.AluOpType

    nc = bacc.Bacc("TRN2", target_bir_lowering=False, debug=False)
    x_d = nc.dram_tensor("x", (C, H, W), f32, kind="ExternalInput")
    wk_d = nc.dram_tensor("wk", (C, K * CO), bf16, kind="ExternalInput")
    pw2t_d = nc.dram_tensor("pw2t", (CO + 1, K), bf16, kind="ExternalInput")
    bh2_d = nc.dram_tensor("bh2", (CO, 1), f32, kind="ExternalInput")
    iden_d = nc.dram_tensor("iden", (C, C), bf16, kind="ExternalInput")
    ones_d = nc.dram_tensor("ones", (1, NQ), bf16, kind="ExternalInput")
    y_d = nc.dram_tensor("y", (W, H * C), bf16, kind="ExternalOutput")

    with ExitStack() as ctx:
        tc = ctx.enter_context(tile.TileContext(nc))
        st = ctx.enter_context(tc.tile_pool(name="st", bufs=1))
        xsp = ctx.enter_context(tc.tile_pool(name="xsp", bufs=2))
        prp = ctx.enter_context(tc.tile_pool(name="prp", bufs=3))
        nump = ctx.enter_context(tc.tile_pool(name="nump", bufs=3))
        h2p = ctx.enter_context(tc.tile_pool(name="h2p", bufs=2, space="PSUM"))
        ltp = ctx.enter_context(tc.tile_pool(name="ltp", bufs=2, space="PSUM"))
        xtp = ctx.enter_context(tc.tile_pool(name="xtp", bufs=4, space="PSUM"))

        x_bf = st.tile([C, XBF], bf16, tag="x_bf")
        h2a = st.tile([CO + 1, NQ], bf16, tag="h2a")
        e_t = st.tile([W, H * K], bf16, tag="e_t")
        kw2 = st.tile([W, H * K * 2], bf16, tag="kw2")   # normalized, dup pairs
        den = st.tile([W, H], f32, tag="den")
        recip = st.tile([W, H], f32, tag="recip")
        x_t = st.tile([W, RS * C], bf16, tag="x_t")
        x_tm1 = st.tile([W, RS * C], bf16, tag="x_tm1")
        x_tp1 = st.tile([W, RS * C], bf16, tag="x_tp1")
        wk_sb = st.tile([C, K * CO], bf16, tag="wk_sb")
        pw2t_sb = st.tile([CO + 1, K], bf16, tag="pw2t_sb")
        bh2_sb = st.tile([CO, 1], f32, tag="bh2_sb")
        iden_sb = st.tile([C, C], bf16, tag="iden_sb")

        # ---- init: params + pad memsets
        nc.gpsimd.dma_start(wk_sb[:], wk_d.ap())
        nc.gpsimd.dma_start(pw2t_sb[:], pw2t_d.ap())
        nc.gpsimd.dma_start(bh2_sb[:], bh2_d.ap())
        nc.gpsimd.dma_start(iden_sb[:], iden_d.ap())
        nc.gpsimd.dma_start(h2a[CO:CO + 1, :], ones_d.ap())
        # x_bf zero pads: head, per-row 2-col gaps, tail
        nc.gpsimd.memset(x_bf[:, 0:PIX0], 0.0)
        nc.gpsimd.memset(
            x_bf[:, PIX0 + W:PIX0 + W + 127 * RS]
            .rearrange("p (g t) -> p g t", t=RS)[:, :, 0:2], 0.0)
        nc.gpsimd.memset(x_bf[:, PIX0 + 127 * RS + W:XBF], 0.0)
        # x_t / shifts: zero pad row-slots 0 and 129; zero edge partitions
        for t in (x_t, x_tm1, x_tp1):
            nc.gpsimd.memset(t[:, 0:C], 0.0)
            nc.gpsimd.memset(t[:, (RS - 1) * C:RS * C], 0.0)

        # ---- PE warm-up: dummy matmuls so HAM reaches 2.4 GHz before the
        # real stream (no data deps; runs during initial DMAs)
        wup = xtp.tile([W, 8 * C], bf16, tag="xt_ps")
        for _ in range(24):
            nc.tensor.transpose(wup[0:C, 0:C], iden_sb[:], iden_sb[:])

        # ---- cursor pipeline with uniform 16-row fronts: B/C advance by
        # exact readiness instead of band-lag, so the DVE starts early.
        def front(h0):
            for g2 in range(2):
                hh = h0 + g2 * 8
                xs = xsp.tile([C, 8 * W], f32, tag="xs")
                nc.sync.dma_start(xs[:], x_d.ap()[:, hh:hh + 8, :])
                dst = (x_bf[:, PIX0 + hh * RS: PIX0 + (hh + 8) * RS]
                       .rearrange("p (h w) -> p h w", w=RS)[:, :, 0:W])
                nc.scalar.copy(dst, xs[:].rearrange("p (h w) -> p h w", w=W))
                # transposes (3 shifts; w+-1 = free-dim offset, pad cols = 0)
                for dst_t, dq in ((x_tm1, -1), (x_t, 0), (x_tp1, 1)):
                    xt_ps = xtp.tile([W, 8 * C], bf16, tag="xt_ps")
                    for r in range(8):
                        q = PIX0 + (hh + r) * RS + dq
                        nc.tensor.transpose(
                            xt_ps[:, r * C:(r + 1) * C],
                            x_bf[:, q:q + W], iden_sb[:])
                    nc.scalar.copy(
                        dst_t[:, (hh + 1) * C:(hh + 9) * C], xt_ps[:])

        def emit_chunk(i):
            q0 = PIX0 + CHUNK * i
            ps = h2p.tile([CO, CHUNK], f32, tag="h2ps")
            for k in range(K):
                delta = (k // 3 - 1) * RS + (k % 3 - 1)
                nc.tensor.matmul(
                    ps[:],
                    lhsT=wk_sb[:, k * CO:(k + 1) * CO],
                    rhs=x_bf[:, q0 + delta:q0 + delta + CHUNK],
                    start=(k == 0), stop=(k == K - 1),
                )
            nc.scalar.activation(
                h2a[0:CO, CHUNK * i:CHUNK * (i + 1)], ps[:],
                AF.Lrelu, bias=bh2_sb[:], scale=1.0, alpha=0.2,
            )

        def emit_b(r0, rn):
            lt = ltp.tile([W, HB * K], f32, tag="lt")
            for r in range(rn):
                h = r0 + r
                nc.tensor.matmul(
                    lt[:, r * K:(r + 1) * K],
                    lhsT=h2a[:, h * RS:h * RS + W],
                    rhs=pw2t_sb[:], start=True, stop=True)
            eb = e_t[:, r0 * K:(r0 + rn) * K]
            nc.scalar.activation(eb, lt[:, 0:rn * K], AF.Exp)
            db = den[:, r0:r0 + rn]
            nc.vector.tensor_reduce(
                db, eb.rearrange("p (h k) -> p h k", k=K),
                axis=mybir.AxisListType.X, op=OP.add)
            rb = recip[:, r0:r0 + rn]
            nc.vector.reciprocal(rb, db)
            nc.vector.tensor_mul(
                kw2[:, r0 * K * 2:(r0 + rn) * K * 2]
                .rearrange("p (h k d) -> p h k d", k=K, d=2),
                eb.rearrange("p (h k) -> p h k", k=K)
                .unsqueeze(3).broadcast_to([W, rn, K, 2]),
                rb.unsqueeze(2).broadcast_to([W, rn, K])
                .unsqueeze(3).broadcast_to([W, rn, K, 2]),
            )

        srcs = {0: x_tm1, 1: x_t, 2: x_tp1}

        def emit_c(r0, rn):
            numt = nump.tile([W, HB * C], bf16, tag="numt")
            accv = numt[:, 0:rn * C]
            for k in range(K):
                i, j = k // 3, k % 3
                xsrc = (srcs[j][:, (r0 + i) * C:(r0 + i + rn) * C]
                        .rearrange("p (h c2 d) -> p h c2 d", c2=C // 2, d=2))
                kwv = (kw2[:, r0 * K * 2:(r0 + rn) * K * 2]
                       .rearrange("p (h k d) -> p h k d", k=K, d=2)[:, :, k, :]
                       .unsqueeze(2).broadcast_to([W, rn, C // 2, 2]))
                if k == 0:
                    nc.vector.tensor_tensor(
                        accv.rearrange("p (h c2 d) -> p h c2 d", c2=C // 2, d=2),
                        xsrc, kwv, op=OP.mult)
                else:
                    prod = prp.tile([W, HB * C], bf16, tag="prod")
                    nc.vector.tensor_tensor(
                        prod[:, 0:rn * C]
                        .rearrange("p (h c2 d) -> p h c2 d", c2=C // 2, d=2),
                        xsrc, kwv, op=OP.mult)
                    nc.vector.tensor_add(accv, accv, prod[:, 0:rn * C])
            nc.sync.dma_start(y_d.ap()[:, r0 * C:(r0 + rn) * C], accv)

        loaded = 0
        next_chunk = 0
        curB = 0
        curC = 0
        for step in range(NB + 3):
            if step < NB:
                front(step * HB)
                loaded = (step + 1) * HB
            while next_chunk < NCHUNK:
                need_row = min(H - 1, (CHUNK * (next_chunk + 1) + RS) // RS)
                if need_row >= loaded and loaded < H:
                    break
                emit_chunk(next_chunk)
                next_chunk += 1
            newB = curB
            while newB < H and (RS * newB + W - 1) // CHUNK < next_chunk:
                newB += 1
            while curB < newB:
                rn = min(HB, newB - curB)
                if rn < 8 and newB < H:
                    break
                emit_b(curB, rn)
                curB += rn
            limit = min(curB, loaded - 1 if loaded < H else H)
            while curC < limit:
                rn = min(HB, limit - curC)
                if rn < 8 and limit < H:
                    break
                emit_c(curC, rn)
                curC += rn
        assert curB == H and curC == H and next_chunk == NCHUNK, \
            (curB, curC, next_chunk)

    nc.compile()
    return nc


def _get_nc():
    if "nc" not in _CACHE:
        _CACHE["nc"] = _build()
    return _CACHE["nc"]


def kernel(x, dw_w, dw_b, pw1_w, pw1_b, pw2_w, pw2_b):
    from concourse.bass_utils import run_bass_kernel_spmd

    x = np.asarray(x, np.float32)
    dw_w = np.asarray(dw_w, np.float32)
    dw_b = np.asarray(dw_b, np.float32)
    pw1_w = np.asarray(pw1_w, np.float32)
    pw1_b = np.asarray(pw1_b, np.float32)
    pw2_w = np.asarray(pw2_w, np.float32)
    pw2_b = np.asarray(pw2_b, np.float32)

    bf = ml_dtypes.bfloat16
    # fused weights: wk[c, k*CO + o] = pw1_w[o, c] * dw_w[c, 0, k//3, k%3]
    wk = np.empty((C, K, CO), np.float32)
    for k in range(K):
        wk[:, k, :] = pw1_w.T * dw_w[:, 0, k // 3, k % 3][:, None]
    wk = wk.reshape(C, K * CO).astype(bf)
    pw2t = np.concatenate([pw2_w.T, pw2_b[None, :]], axis=0).astype(bf)
    bh2 = (pw1_w @ dw_b + pw1_b).reshape(CO, 1).astype(np.float32)
    iden = np.eye(C, dtype=np.float32).astype(bf)
    ones = np.ones((1, NQ), np.float32).astype(bf)

    nc = _get_nc()
    in_maps = [
        {"x": np.ascontiguousarray(x[b]), "wk": wk, "pw2t": pw2t,
         "bh2": bh2, "iden": iden, "ones": ones}
        for b in range(8)
    ]
    res = run_bass_kernel_spmd(nc, in_maps, core_ids=list(range(8)),
                               **_CACHE.get("run_kwargs", {}))
    _CACHE["last_result"] = res
    out = np.empty((8, C, H, W), np.float32)
    for b in range(8):
        out[b] = res.results[b]["y"].astype(np.float32).reshape(W, H, C).transpose(2, 1, 0)
    return out


# revision 34
# speedup vs baseline: 1.0429x; 1.0429x over previous
"""AdaptiveLowPassFilter Trainium2 kernel — 8-core batch-parallel.

Per core (one image, x [96,128,128] f32):
  phase A  (PE):  fused depthwise3x3+pw1 as 9 shifted matmuls accumulating in
                  PSUM (channels on partitions, padded flat pixels free; all
                  9 taps are pure free-dim offsets).
  leaky    (ACT): bias + LeakyReLU(0.2) from PSUM into h2 (bf16).
  phase B  (PE):  pw2 computed TRANSPOSED per image row: lhsT=h2[:,row],
                  rhs=pw2^T (aug with bias row) -> logits land pixel-major
                  [w, 9] in PSUM; ACT exp -> unnormalized e_t.
  softmax  (DVE): den = reduce over 9 taps, reciprocal; kw2 = e*recip written
                  as duplicated PAIRS (so phase C multiplies hit the DVE
                  2x_1P mode: innermost AP dim is a step-1 bf16 pair).
  x_t      (PE):  per-row transposes of x into pixel-major [w, (h, c)];
                  w+-1 taps via partition-shifted SBUF->SBUF band DMAs.
  phase C  (DVE): per 16-row band: 9 big tensor_tensor pair-multiplies +
                  8 big adds: num[w,(h,c)] = sum_k kw[w,h,k]*x_t{shift}[...].
  out      (ACT): bf16->f32, DMA out as [w, (h, c)]; host transposes.

All phases are banded (16 rows) so Tile can pipeline engines end-to-end.
"""
import sys
sys.path.insert(0, "/opt/trn_rl_repo")

import numpy as np
import ml_dtypes
from contextlib import ExitStack

C, CO, H, W, K = 96, 48, 128, 128, 9
RS = 130            # padded row stride in flat pixel space
PIX0 = 131          # flat offset of pixel (0, 0)
XBF = 17160         # x_bf free size
NQ = 16896          # 33 chunks x 512 of h2 pixel space
NCHUNK = 33
CHUNK = 512
HB = 16             # band height (rows)
NB = H // HB        # 8 bands

_CACHE = {}


def _build():
    import concourse.bass as bass
    import concourse.bacc as bacc
    import concourse.tile as tile
    import concourse.mybir as mybir

    dt = mybir.dt
    f32, bf16 = dt.float32, dt.bfloat16
    AF = mybir.ActivationFunctionType
    OP = mybir.AluOpType

    nc = bacc.Bacc("TRN2", target_bir_lowering=False, debug=False)
    x_d = nc.dram_tensor("x", (C, H, W), f32, kind="ExternalInput")
    wk_d = nc.dram_tensor("wk", (C, K * CO), bf16, kind="ExternalInput")
    pw2t_d = nc.dram_tensor("pw2t", (CO + 1, K), bf16, kind="ExternalInput")
    bh2_d = nc.dram_tensor("bh2", (CO, 1), f32, kind="ExternalInput")
    iden_d = nc.dram_tensor("iden", (C, C), bf16, kind="ExternalInput")
    ones_d = nc.dram_tensor("ones", (1, NQ), bf16, kind="ExternalInput")
    y_d = nc.dram_tensor("y", (W, H * C), bf16, kind="ExternalOutput")

    with ExitStack() as ctx:
        tc = ctx.enter_context(tile.TileContext(nc))
        st = ctx.enter_context(tc.tile_pool(name="st", bufs=1))
        xsp = ctx.enter_context(tc.tile_pool(name="xsp", bufs=2))
        prp = ctx.enter_context(tc.tile_pool(name="prp", bufs=3))
        nump = ctx.enter_context(tc.tile_pool(name="nump", bufs=3))
        h2p = ctx.enter_context(tc.tile_pool(name="h2p", bufs=2, space="PSUM"))
        ltp = ctx.enter_context(tc.tile_pool(name="ltp", bufs=2, space="PSUM"))
        xtp = ctx.enter_context(tc.tile_pool(name="xtp", bufs=4, space="PSUM"))

        x_bf = st.tile([C, XBF], bf16, tag="x_bf")
        h2a = st.tile([CO + 1, NQ], bf16, tag="h2a")
        e_t = st.tile([W, H * K], bf16, tag="e_t")
        kw2 = st.tile([W, H * K * 2], bf16, tag="kw2")   # normalized, dup pairs
        den = st.tile([W, H], f32, tag="den")
        recip = st.tile([W, H], f32, tag="recip")
        x_t = st.tile([W, RS * C], bf16, tag="x_t")
        x_tm1 = st.tile([W, RS * C], bf16, tag="x_tm1")
        x_tp1 = st.tile([W, RS * C], bf16, tag="x_tp1")
        wk_sb = st.tile([C, K * CO], bf16, tag="wk_sb")
        pw2t_sb = st.tile([CO + 1, K], bf16, tag="pw2t_sb")
        bh2_sb = st.tile([CO, 1], f32, tag="bh2_sb")
        iden_sb = st.tile([C, C], bf16, tag="iden_sb")

        # ---- init: params + pad memsets
        nc.scalar.dma_start(wk_sb[:], wk_d.ap())
        nc.scalar.dma_start(pw2t_sb[:], pw2t_d.ap())
        nc.scalar.dma_start(bh2_sb[:], bh2_d.ap())
        nc.scalar.dma_start(iden_sb[:], iden_d.ap())
        nc.scalar.dma_start(h2a[CO:CO + 1, :], ones_d.ap())
        # x_bf zero pads: head, per-row 2-col gaps, tail
        nc.gpsimd.memset(x_bf[:, 0:PIX0], 0.0)
        nc.gpsimd.memset(
            x_bf[:, PIX0 + W:PIX0 + W + 127 * RS]
            .rearrange("p (g t) -> p g t", t=RS)[:, :, 0:2], 0.0)
        nc.gpsimd.memset(x_bf[:, PIX0 + 127 * RS + W:XBF], 0.0)
        # x_t / shifts: zero pad row-slots 0 and 129; zero edge partitions
        for t in (x_t, x_tm1, x_tp1):
            nc.gpsimd.memset(t[:, 0:C], 0.0)
            nc.gpsimd.memset(t[:, (RS - 1) * C:RS * C], 0.0)

        # ---- PE warm-up: dummy matmuls so HAM reaches 2.4 GHz before the
        # real stream (no data deps; runs during initial DMAs)
        wup = xtp.tile([W, 8 * C], bf16, tag="xt_ps")
        for _ in range(24):
            nc.tensor.transpose(wup[0:C, 0:C], iden_sb[:], iden_sb[:])

        # ---- cursor pipeline with uniform 16-row fronts: B/C advance by
        # exact readiness instead of band-lag, so the DVE starts early.
        def front(h0):
            for g2 in range(2):
                hh = h0 + g2 * 8
                xs = xsp.tile([C, 8 * W], f32, tag="xs")
                nc.sync.dma_start(xs[:], x_d.ap()[:, hh:hh + 8, :])
                dst = (x_bf[:, PIX0 + hh * RS: PIX0 + (hh + 8) * RS]
                       .rearrange("p (h w) -> p h w", w=RS)[:, :, 0:W])
                nc.scalar.copy(dst, xs[:].rearrange("p (h w) -> p h w", w=W))
                # transposes (3 shifts; w+-1 = free-dim offset, pad cols = 0)
                for dst_t, dq in ((x_tm1, -1), (x_t, 0), (x_tp1, 1)):
                    xt_ps = xtp.tile([W, 8 * C], bf16, tag="xt_ps")
                    for r in range(8):
                        q = PIX0 + (hh + r) * RS + dq
                        nc.tensor.transpose(
                            xt_ps[:, r * C:(r + 1) * C],
                            x_bf[:, q:q + W], iden_sb[:])
                    nc.scalar.copy(
                        dst_t[:, (hh + 1) * C:(hh + 9) * C], xt_ps[:])

        def emit_chunk(i):
            q0 = PIX0 + CHUNK * i
            ps = h2p.tile([CO, CHUNK], f32, tag="h2ps")
            for k in range(K):
                delta = (k // 3 - 1) * RS + (k % 3 - 1)
                nc.tensor.matmul(
                    ps[:],
                    lhsT=wk_sb[:, k * CO:(k + 1) * CO],
                    rhs=x_bf[:, q0 + delta:q0 + delta + CHUNK],
                    start=(k == 0), stop=(k == K - 1),
                )
            nc.scalar.activation(
                h2a[0:CO, CHUNK * i:CHUNK * (i + 1)], ps[:],
                AF.Lrelu, bias=bh2_sb[:], scale=1.0, alpha=0.2,
            )

        def emit_b(r0, rn):
            lt = ltp.tile([W, HB * K], f32, tag="lt")
            for r in range(rn):
                h = r0 + r
                nc.tensor.matmul(
                    lt[:, r * K:(r + 1) * K],
                    lhsT=h2a[:, h * RS:h * RS + W],
                    rhs=pw2t_sb[:], start=True, stop=True)
            eb = e_t[:, r0 * K:(r0 + rn) * K]
            nc.scalar.activation(eb, lt[:, 0:rn * K], AF.Exp)
            db = den[:, r0:r0 + rn]
            nc.vector.tensor_reduce(
                db, eb.rearrange("p (h k) -> p h k", k=K),
                axis=mybir.AxisListType.X, op=OP.add)
            rb = recip[:, r0:r0 + rn]
            nc.vector.reciprocal(rb, db)
            nc.vector.tensor_mul(
                kw2[:, r0 * K * 2:(r0 + rn) * K * 2]
                .rearrange("p (h k d) -> p h k d", k=K, d=2),
                eb.rearrange("p (h k) -> p h k", k=K)
                .unsqueeze(3).broadcast_to([W, rn, K, 2]),
                rb.unsqueeze(2).broadcast_to([W, rn, K])
                .unsqueeze(3).broadcast_to([W, rn, K, 2]),
            )

        srcs = {0: x_tm1, 1: x_t, 2: x_tp1}

        def emit_c(r0, rn):
            numt = nump.tile([W, HB * C], bf16, tag="numt")
            accv = numt[:, 0:rn * C]
            for k in range(K):
                i, j = k // 3, k % 3
                xsrc = (srcs[j][:, (r0 + i) * C:(r0 + i + rn) * C]
                        .rearrange("p (h c2 d) -> p h c2 d", c2=C // 2, d=2))
                kwv = (kw2[:, r0 * K * 2:(r0 + rn) * K * 2]
                       .rearrange("p (h k d) -> p h k d", k=K, d=2)[:, :, k, :]
                       .unsqueeze(2).broadcast_to([W, rn, C // 2, 2]))
                if k == 0:
                    nc.vector.tensor_tensor(
                        accv.rearrange("p (h c2 d) -> p h c2 d", c2=C // 2, d=2),
                        xsrc, kwv, op=OP.mult)
                else:
                    prod = prp.tile([W, HB * C], bf16, tag="prod")
                    nc.vector.tensor_tensor(
                        prod[:, 0:rn * C]
                        .rearrange("p (h c2 d) -> p h c2 d", c2=C // 2, d=2),
                        xsrc, kwv, op=OP.mult)
                    nc.vector.tensor_add(accv, accv, prod[:, 0:rn * C])
            nc.sync.dma_start(y_d.ap()[:, r0 * C:(r0 + rn) * C], accv)

        loaded = 0
        next_chunk = 0
        curB = 0
        curC = 0
        for step in range(NB + 3):
            if step < NB:
                front(step * HB)
                loaded = (step + 1) * HB
            while next_chunk < NCHUNK:
                need_row = min(H - 1, (CHUNK * (next_chunk + 1) + RS) // RS)
                if need_row >= loaded and loaded < H:
                    break
                emit_chunk(next_chunk)
                next_chunk += 1
            newB = curB
            while newB < H and (RS * newB + W - 1) // CHUNK < next_chunk:
                newB += 1
            while curB < newB:
                rn = min(HB, newB - curB)
                if rn < 8 and newB < H:
                    break
                emit_b(curB, rn)
                curB += rn
            limit = min(curB, loaded - 1 if loaded < H else H)
            while curC < limit:
                rn = min(HB, limit - curC)
                if rn < 8 and limit < H:
                    break
                emit_c(curC, rn)
                curC += rn
        assert curB == H and curC == H and next_chunk == NCHUNK, \
            (curB, curC, next_chunk)

    nc.compile()
    return nc


def _get_nc():
    if "nc" not in _CACHE:
        _CACHE["nc"] = _build()
    return _CACHE["nc"]


def kernel(x, dw_w, dw_b, pw1_w, pw1_b, pw2_w, pw2_b):
    from concourse.bass_utils import run_bass_kernel_spmd

    x = np.asarray(x, np.float32)
    dw_w = np.asarray(dw_w, np.float32)
    dw_b = np.asarray(dw_b, np.float32)
    pw1_w = np.asarray(pw1_w, np.float32)
    pw1_b = np.asarray(pw1_b, np.float32)
    pw2_w = np.asarray(pw2_w, np.float32)
    pw2_b = np.asarray(pw2_b, np.float32)

    bf = ml_dtypes.bfloat16
    # fused weights: wk[c, k*CO + o] = pw1_w[o, c] * dw_w[c, 0, k//3, k%3]
    wk = np.empty((C, K, CO), np.float32)
    for k in range(K):
        wk[:, k, :] = pw1_w.T * dw_w[:, 0, k // 3, k % 3][:, None]
    wk = wk.reshape(C, K * CO).astype(bf)
    pw2t = np.concatenate([pw2_w.T, pw2_b[None, :]], axis=0).astype(bf)
    bh2 = (pw1_w @ dw_b + pw1_b).reshape(CO, 1).astype(np.float32)
    iden = np.eye(C, dtype=np.float32).astype(bf)
    ones = np.ones((1, NQ), np.float32).astype(bf)

    nc = _get_nc()
    in_maps = [
        {"x": np.ascontiguousarray(x[b]), "wk": wk, "pw2t": pw2t,
         "bh2": bh2, "iden": iden, "ones": ones}
        for b in range(8)
    ]
    res = run_bass_kernel_spmd(nc, in_maps, core_ids=list(range(8)),
                               **_CACHE.get("run_kwargs", {}))
    _CACHE["last_result"] = res
    out = np.empty((8, C, H, W), np.float32)
    for b in range(8):
        out[b] = res.results[b]["y"].astype(np.float32).reshape(W, H, C).transpose(2, 1, 0)
    return out
